# revision 4
# baseline (speedup 1.0000x reference)
"""GAT message-passing kernel v2 for 8 Trainium2 NeuronCores.

Design (edge-parallel by dst-range, no collectives):
  Host: sort edges by dst; core c owns dst nodes [c*12500, (c+1)*12500).
  Within a core, dst tiles of 128 nodes; edges of a tile are split by src
  bank (4 banks of 25600 rows, int16-indexable) into static chunk counts
  g[t][b] = max over cores of ceil(count/128).

  Gather: dma_gather (SWDGE Q7 batch gather), one call per (supertile of 2
  dst tiles, bank), ~1024 idxs/call. Pads cycle rows 1..128 (constant-value
  pad runs wedge the SDMA; padded rows are masked out downstream). Output
  [128, chunks, 64] f32: edge j of a call lands at (partition j%128,
  col j//128).

  Compute per chunk of 128 edges (dst tile T):
    PE transpose gather slice [128e,64] -> hkT psum (f32), pairs share one
      [128,256] DVE copy -> fp16 megapair (chunk A partitions 0:64, B 64:128)
    mm1a: st[e,d] = hkT^T @ hut_tile (fp16, f32 psum)
    mm1b: += maskT_chunk^T @ maskd  (bit-match mask fold: rows 0:7 C*bit_b(
      ldst), 7:14 C*(1-bit_b); matching dst -> +7C, else <= +6C)
    exp (per pair): pt = exp(st - 7C) bf16  -> masked softmax numerators
    vals: copy gather slice f32->bf16 into ring tile with ones col 64
    mm2: rst[128d, 65] += pt^T @ vals  (col 64 = denominator)
  Epilogue per tile: den+=eps, recip, transpose rst, FC matmul with
  [W^T; b] (bias row scales by den so normalize folds after FC), ACT
  relu(out * recip), DMA out.
"""
import contextlib
import sys

for p in ("/opt/trn_rl_repo",):
    if p not in sys.path:
        sys.path.insert(0, p)

import numpy as np
import concourse.bass as bass
import concourse.tile as tile
from concourse import mybir, bacc
from concourse.bass_utils import run_bass_kernel_spmd
from concourse.masks import make_identity

f32 = mybir.dt.float32
f16 = mybir.dt.float16
bf16 = mybir.dt.bfloat16
i16 = mybir.dt.int16

N_CORES = 8
P = 128
N_NODES = 100000
D_FEAT = 64
D_OUT = 128
NPC = N_NODES // N_CORES          # 12500
N_TILES = (NPC + P - 1) // P      # 98
PAD_NODES = N_TILES * P           # 12544
BANKS = 4
BANK_SZ = 25600
ST = 3                            # dst tiles per gather supertile
N_ST = (N_TILES + ST - 1) // ST   # 49
C_MASK = 100.0


def build_v2(g, repeat=1, ablate=None):
    """g: [N_TILES][BANKS] static chunk counts (python ints)."""
    g = [[int(x) for x in row] for row in g]
    tile_chunks = [sum(row) for row in g]
    totc = sum(tile_chunks)
    max_tc = max(tile_chunks)
    # per-bank max chunks within any supertile call
    call_chunks = [[sum(g[t][b] for t in range(ST * s,
                                               min(ST * s + ST, N_TILES)))
                    for b in range(BANKS)] for s in range(N_ST)]
    maxcall = [max(call_chunks[s][b] for s in range(N_ST))
               for b in range(BANKS)]
    tot_idx = sum(call_chunks[s][b] * P for s in range(N_ST)
                  for b in range(BANKS))

    nc = bacc.Bacc("TRN2", target_bir_lowering=False, debug=False,
                   num_devices=N_CORES)
    hkov = nc.dram_tensor("hkov", [N_NODES, 2 * D_FEAT], f16,
                          kind="ExternalInput")
    idxs = nc.dram_tensor("idxs", [P, tot_idx // 16], i16,
                          kind="ExternalInput")
    maskt = nc.dram_tensor("maskt", [14, tot_idx], f16, kind="ExternalInput")
    hut = nc.dram_tensor("hut", [D_FEAT + 14, PAD_NODES], f16,
                         kind="ExternalInput")
    wtaug = nc.dram_tensor("wtaug", [D_FEAT + 1, D_OUT], bf16,
                           kind="ExternalInput")
    y = nc.dram_tensor("y", [NPC, D_OUT], f32, kind="ExternalOutput")

    with tile.TileContext(nc) as tc:
        with (
            tc.tile_pool(name="const", bufs=1) as cpool,
            tc.tile_pool(name="epi", bufs=2) as epool,
            tc.tile_pool(name="ps_tr", bufs=2, space="PSUM") as ps_tr,
            tc.tile_pool(name="ps_st", bufs=2, space="PSUM") as ps_st,
            tc.tile_pool(name="ps_rst", bufs=2, space="PSUM") as ps_rst,
            tc.tile_pool(name="ps_epi", bufs=1, space="PSUM") as ps_epi,
        ):
            ident = cpool.tile([P, P], f32)
            make_identity(nc, ident[:])
            identh = cpool.tile([P, P], f16)
            make_identity(nc, identh[:])
            hut_sb = cpool.tile([D_FEAT + 14, PAD_NODES], f16)
            nc.sync.dma_start(hut_sb[:], hut.ap())
            wt_sb = cpool.tile([D_FEAT + 1, D_OUT], bf16)
            nc.sync.dma_start(wt_sb[:], wtaug.ap())
            idx_sb = cpool.tile([P, tot_idx // 16], i16)
            nc.sync.dma_start(idx_sb[:], idxs.ap())
            bias_t = cpool.tile([P, 1], f32)
            nc.vector.memset(bias_t[:], -7.0 * C_MASK)

            # manual rings (memset once -> no stale-NaN on skipped slots)
            NG = 3
            G = [[cpool.tile([P, maxcall[b] * P], f16,
                             tag=f"G{r}b{b}", name=f"G{r}b{b}")
                  for b in range(BANKS)] for r in range(NG)]
            for r in range(NG):
                for b in range(BANKS):
                    nc.vector.memset(G[r][b][:], 0.0)

            NVAL = 6
            V = [cpool.tile([P, D_FEAT + 1], bf16, tag=f"V{i}",
                            name=f"V{i}") for i in range(NVAL)]
            for i in range(NVAL):
                nc.vector.memset(V[i][:], 1.0)

            NPT = 3
            PT = [cpool.tile([P, 2 * P], bf16, tag=f"PT{i}",
                             name=f"PT{i}") for i in range(NPT)]
            GFAKE = None
            if ablate == "decoupled":
                GFAKE = cpool.tile([P, 16 * D_FEAT], f32, name="GFAKE")
                nc.vector.memset(GFAKE[:], 0.01)

            idx_off = 0        # in idx columns (16 idxs per column)
            call_off = [[0] * BANKS for _ in range(N_ST)]
            o = 0
            for s in range(N_ST):
                for b in range(BANKS):
                    call_off[s][b] = o
                    o += call_chunks[s][b] * P // 16

            vi = [0]           # vals ring cursor
            mi = [0]           # mega ring cursor
            pi = [0]           # pt ring cursor

            def do_tile(T, s):
                """Process dst tile T inside supertile s."""
                rst = ps_rst.tile([P, D_FEAT + 1], f32, tag="rst")
                # chunk list (bank-major); pairs share one exp
                chunks = []
                for b in range(BANKS):
                    base = sum(g[t2][b] for t2 in range(ST * s, T))
                    for c in range(g[T][b]):
                        chunks.append((b, base + c))
                n_ch = len(chunks)
                for i0 in range(0, n_ch, 2):
                    pair = chunks[i0:i0 + 2]
                    npair = len(pair)
                    st_ps = ps_st.tile([P, 2 * P], f32, tag="st")
                    for k, (b2, col) in enumerate(pair):
                        gt = G[s % NG][b2]
                        nc.tensor.matmul(
                            out=st_ps[:, k * P:(k + 1) * P],
                            lhsT=gt[0:78, col * P:(col + 1) * P],
                            rhs=hut_sb[:, T * P:(T + 1) * P],
                            start=True, stop=True)
                    pt = PT[pi[0] % NPT]; pi[0] += 1
                    nc.scalar.activation(pt[:, 0:npair * P],
                                         st_ps[:, 0:npair * P],
                                         mybir.ActivationFunctionType.Exp,
                                         bias=bias_t[:])
                    for k, (b2, col) in enumerate(pair):
                        cc = i0 + k
                        gt = G[s % NG][b2]
                        tv = ps_tr.tile([P, P], f16, tag="tr")
                        nc.tensor.transpose(
                            out=tv[:], in_=gt[:, col * P:(col + 1) * P],
                            identity=identh[:])
                        v = V[vi[0] % NVAL]; vi[0] += 1
                        if cc % 2 == 0:
                            nc.vector.tensor_copy(out=v[:, 0:D_FEAT],
                                                  in_=tv[:, 0:D_FEAT])
                        else:
                            nc.scalar.copy(out=v[:, 0:D_FEAT],
                                           in_=tv[:, 0:D_FEAT])
                        nc.tensor.matmul(out=rst[:],
                                         lhsT=pt[:, k * P:(k + 1) * P],
                                         rhs=v[:], start=(cc == 0),
                                         stop=(cc == n_ch - 1))
                # epilogue
                rst_sb = epool.tile([P, D_FEAT + 1], f32, tag="rst_sb")
                nc.vector.tensor_copy(out=rst_sb[:], in_=rst[:])
                nc.vector.tensor_scalar_add(rst_sb[:, D_FEAT:D_FEAT + 1],
                                            rst_sb[:, D_FEAT:D_FEAT + 1],
                                            1e-30)
                recip = epool.tile([P, 1], f32, tag="recip")
                nc.vector.reciprocal(recip[:], rst_sb[:, D_FEAT:D_FEAT + 1])
                rstT_ps = ps_epi.tile([D_FEAT + 1, P], f32, tag="rstT")
                nc.tensor.transpose(out=rstT_ps[:], in_=rst_sb[:],
                                    identity=ident[:])
                rstT = epool.tile([D_FEAT + 1, P], bf16, tag="rstT_sb")
                nc.vector.tensor_copy(out=rstT[:], in_=rstT_ps[:])
                out_ps = ps_epi.tile([P, D_OUT], f32, tag="out_ps")
                nc.tensor.matmul(out=out_ps[:], lhsT=rstT[:], rhs=wt_sb[:],
                                 start=True, stop=True)
                out_sb = epool.tile([P, D_OUT], f32, tag="out_sb")
                nc.scalar.activation(out_sb[:], out_ps[:],
                                     mybir.ActivationFunctionType.Relu,
                                     scale=recip[:])
                rows = min(P, NPC - T * P)
                nc.sync.dma_start(y.ap()[T * P:T * P + rows], out_sb[:rows])

            if True:
                pass
            loop_cm = (tc.For_i(0, repeat, 1) if repeat > 1
                       else contextlib.nullcontext())
            def emit_gathers(s):
                if ablate in ("no_gather", "decoupled_ng"):
                    return
                for b in range(BANKS):
                    nch = call_chunks[s][b]
                    if nch == 0:
                        continue
                    gt = G[s % NG][b]
                    nc.gpsimd.dma_gather(
                        out_ap=gt[:, 0:nch * P].rearrange(
                            "p (o c) -> p o c", o=1),
                        in_ap=hkov.ap()[b * BANK_SZ:
                                        min((b + 1) * BANK_SZ, N_NODES)],
                        idxs_ap=idx_sb[:, call_off[s][b]:
                                       call_off[s][b] + nch * P // 16],
                        num_idxs=nch * P,
                        num_idxs_reg=nch * P,
                        elem_size=2 * D_FEAT,
                        transpose=True,
                        single_packet=False,
                    )
                    nc.sync.dma_start(
                        gt[64:78, 0:nch * P],
                        maskt.ap()[:, call_off[s][b] * 16:
                                   call_off[s][b] * 16 + nch * P])

            with loop_cm:
                # software pipeline: emit gathers one supertile ahead so
                # coarse (tick-based) WAR waits land on compute(s-1), letting
                # gathers(s+1) run during compute(s).
                emit_gathers(0)
                emit_gathers(1)
                for s in range(N_ST):
                    if s + 2 < N_ST:
                        emit_gathers(s + 2)
                    for t in range(ST):
                        T = ST * s + t
                        if T < N_TILES and ablate != "gather_only":
                            do_tile(T, s)
                if ablate == "gather_only":
                    snk = epool.tile([P, D_FEAT], f32, tag="snk")
                    nc.vector.tensor_copy(out=snk[:], in_=G[0][0][:, 0:D_FEAT])
                    nc.sync.dma_start(y.ap()[0:P, 0:D_FEAT], snk[:])
    nc.compile()
    return nc


def prep_inputs_v2(hk, hu, W, b, src, dst):
    """Host-side sharding prep. Returns (per-core in_maps, g, meta)."""
    src = np.asarray(src).astype(np.int64)
    dst = np.asarray(dst).astype(np.int64)
    hk = np.ascontiguousarray(hk, np.float32)
    hu = np.asarray(hu, np.float32)

    order = np.argsort(dst, kind="stable")
    src_s = src[order]
    dst_s = dst[order]
    core = dst_s // NPC
    local = dst_s - core * NPC
    tl = local // P
    ldst = (local % P).astype(np.int64)
    bank = src_s // BANK_SZ
    lidx = (src_s - bank * BANK_SZ).astype(np.int64)

    key = (core * N_TILES + tl) * BANKS + bank
    ord2 = np.argsort(key, kind="stable")
    key_s = key[ord2]
    lidx_s = lidx[ord2]
    ldst_s = ldst[ord2]

    counts = np.bincount(key_s, minlength=N_CORES * N_TILES * BANKS)
    counts = counts.reshape(N_CORES, N_TILES, BANKS)
    g = np.maximum(1, -(-counts.max(axis=0) // P))       # [N_TILES, BANKS]
    tile_chunks = g.sum(axis=1)                          # [N_TILES]
    totc = int(tile_chunks.sum())

    # supertile call sizes and in-call tile segment bases
    call_chunks = np.zeros((N_ST, BANKS), np.int64)
    for s in range(N_ST):
        for t in range(ST * s, min(ST * s + ST, N_TILES)):
            call_chunks[s] += g[t]
    tot_idx = int(call_chunks.sum() * P)

    # per-edge slot: position j within its (core,tile,bank) run
    starts = np.zeros(N_CORES * N_TILES * BANKS + 1, np.int64)
    np.cumsum(counts.reshape(-1), out=starts[1:])
    j_in_run = np.arange(len(key_s)) - starts[key_s]

    t_of = (key_s // BANKS) % N_TILES
    b_of = key_s % BANKS
    c_of = key_s // (N_TILES * BANKS)

    # in-call idx position: tile segment base + j
    s_of = t_of // ST
    # exclusive prefix of g within each supertile block
    gcs = np.cumsum(g, axis=0)
    block0 = (np.arange(N_TILES) // ST) * ST
    prefix = np.zeros_like(g)
    for t in range(N_TILES):
        if t > block0[t]:
            prefix[t] = gcs[t - 1] - (gcs[block0[t] - 1]
                                      if block0[t] > 0 else 0)
    seg_base = prefix[t_of, b_of] * P
    call_pos = seg_base + j_in_run
    # call base offset in the global idx stream (st-major, bank minor)
    call_base = np.concatenate(
        [[0], np.cumsum(call_chunks.reshape(-1) * P)])[:-1].reshape(
        N_ST, BANKS)
    idx_pos = call_base[s_of, b_of] + call_pos
    mask_pos = idx_pos          # mask stream is call-ordered now

    # ---- build per-core arrays
    in_maps = []
    maskd = np.zeros((14, P), np.float16)
    d_ar = np.arange(P)
    for bb in range(7):
        maskd[bb] = ((d_ar >> bb) & 1).astype(np.float16)
        maskd[7 + bb] = 1.0 - maskd[bb]
    wtaug = np.concatenate([W.T, b[None, :]], axis=0)
    import ml_dtypes
    wtaug = wtaug.astype(ml_dtypes.bfloat16)
    hkf = hk.astype(np.float16)
    hkov = np.zeros((N_NODES, 2 * D_FEAT), np.float16)
    hkov[:, :D_FEAT] = hkf
    hkov[:-1, D_FEAT:] = hkf[1:]

    # idx stream template: ALL pads are varied safe indices. Long runs of a
    # constant pad value (0 or -1) wedge the SDMA engines (HW-observed), so
    # pads cycle through rows 1..128; they gather junk that the zero mask
    # rows suppress.
    idx_template = ((np.arange(tot_idx) % P) + 1).astype(np.int16)

    for c in range(N_CORES):
        m = c_of == c
        idx_arr = idx_template.copy()
        idx_arr[idx_pos[m]] = lidx_s[m].astype(np.int16)
        # wrap [16, n/16] per call then concat: since calls are contiguous
        # 128-multiples, a global wrap of each call block:
        wrapped = np.empty((16, tot_idx // 16), np.int16)
        o16 = 0
        for s in range(N_ST):
            for bb in range(BANKS):
                n = int(call_chunks[s, bb]) * P
                blk = idx_arr[call_base[s, bb]:call_base[s, bb] + n]
                wrapped[:, o16:o16 + n // 16] = blk.reshape(-1, 16).T
                o16 += n // 16
        idx_full = np.tile(wrapped, (8, 1))

        maskt = np.zeros((14, tot_idx), np.float16)
        pos = mask_pos[m]
        ld = ldst_s[m]
        for bb in range(7):
            bit = ((ld >> bb) & 1).astype(np.float16)
            maskt[bb, pos] = C_MASK * bit
            maskt[7 + bb, pos] = C_MASK * (1.0 - bit)

        hut = np.zeros((D_FEAT + 14, PAD_NODES), np.float16)
        hut[:D_FEAT, :NPC] = hu[c * NPC:(c + 1) * NPC].T
        hut[D_FEAT:] = np.tile(maskd, (1, N_TILES))

        in_maps.append({
            "hkov": hkov, "idxs": idx_full, "maskt": maskt, "hut": hut,
            "wtaug": wtaug,
        })
    meta = dict(g=g, call_chunks=call_chunks, totc=totc)
    return in_maps, g, meta


_KERNEL_CACHE = {}


def run_gat(hk, hu, W, b, src, dst, repeat=1):
    in_maps, g, meta = prep_inputs_v2(hk, hu, W, b, src, dst)
    key = (tuple(map(tuple, g)), repeat)
    if key not in _KERNEL_CACHE:
        _KERNEL_CACHE[key] = build_v2(g, repeat=repeat)
    nc = _KERNEL_CACHE[key]
    res = run_bass_kernel_spmd(nc, in_maps, core_ids=list(range(N_CORES)))
    out = np.concatenate([res.results[c]["y"] for c in range(N_CORES)],
                         axis=0)
    return np.ascontiguousarray(out, np.float32)


def kernel(hk, hu, W, b, src, dst):
    return run_gat(np.asarray(hk, np.float32), np.asarray(hu, np.float32),
                   np.asarray(W, np.float32), np.asarray(b, np.float32),
                   np.asarray(src), np.asarray(dst))
